# revision 37
# baseline (speedup 1.0000x reference)
"""Trainium2 Bass kernel for C2AttentionBlock (windowed cross-attention, 2 branches).

Sharding: data-parallel over batch. Core b handles batch image b (64 windows of
64 tokens, C=256). All compute in bf16 matmuls / f32 accumulation.

Device layout (per core, NW windows, T = 64*NW tokens):
  rT/gT   feature-major [256, T] bf16   (2 partition tiles of 128 channels)
  rtok/gtok token-major [128, NW*128] bf16:
            partition p = 64*(c//128) + token, free = 128*w + (c%128)
  Window pairs: scores S and S.T as tile_position-packed K=32/M=64 matmuls
  into 4 PSUM banks (one per PE row-group -- concurrent matmuls that share a
  col-group but differ in row-group must not share a PSUM bank).
  Softmax for BOTH branches from one scores set: denominators via a
  block-diag-ones matmul (64-partition column sums broadcast to all 128
  partitions), then en = e * exp(-ln(rowsum)) -- ACT stays on the single
  natural_log_exp table set (table swaps cost ~1.3-2.7us each).
  AV: 8-way packed K=64 matmuls; O.T assembled feature-major in 2 banks.
  MLP: fc1 (C->4C, exact GELU, gelu table set) + fc2 feature-major;
  LayerNorm feature-major: channel sums via all-ones matmul, rsqrt via a
  DVE Newton step (keeps ACT off the exp set during the gelu phase).
  All elementwise in bf16 where possible (DVE 2X mode); outputs bf16.
"""

import math

import numpy as np

EMBED = 256
HEADS = 8
HD = 32
WS = 8
LN_EPS = 1e-5
N = 64  # tokens per window

_CACHE = {}


def _split_waits(nc, max_waits=1):
    """This walrus build only supports one sync-wait slot per instruction;
    move excess waits onto preceding same-engine NOPs."""
    import concourse.mybir as mybir
    for f in nc.m.functions:
        for blk in f.blocks:
            il = blk.instructions
            out = []
            for inst in il:
                si = inst.sync_info
                if si is not None and len(si.on_wait) > max_waits:
                    waits = list(si.on_wait)
                    for k, w in enumerate(waits[:-max_waits]):
                        nop = mybir.InstNoOp(
                            name=f"{inst.name}-ws{k}",
                            sync_info=mybir.SyncInfo(on_wait=[w], on_update=[]),
                            bass_nofuse=True,
                            engine=inst.engine,
                        )
                        out.append(nop)
                    inst.sync_info = mybir.SyncInfo(
                        on_wait=waits[-max_waits:],
                        on_update=list(si.on_update))
                out.append(inst)
            il.clear()
            il.extend(out)


def _act_recip(nc, mybir, out, in_):
    """ACT-engine reciprocal via raw InstActivation (the bass wrapper bans it
    for accuracy; ~1e-3 rel is fine for softmax denominators here)."""
    eng = nc.scalar
    ins = [eng.lower_ap(in_),
           mybir.ImmediateValue(dtype=mybir.dt.float32, value=0.0),
           mybir.ImmediateValue(dtype=mybir.dt.float32, value=1.0),
           mybir.ImmediateValue(dtype=mybir.dt.float32, value=0.0)]
    return eng.add_instruction(mybir.InstActivation(
        name=nc.get_next_instruction_name(),
        func=mybir.ActivationFunctionType.Reciprocal,
        ins=ins, outs=[eng.lower_ap(out)]))


# ---------------------------------------------------------------- builder
def build_nc(NW=64, split_waits=True):
    import concourse.bass as bass
    import concourse.mybir as mybir
    import concourse.tile as tile

    T = N * NW
    WG = min(8, NW)          # windows per group / chunk
    NWG = NW // WG           # number of groups (== token chunks)
    CH = WG * N              # tokens per chunk (512 normally)
    assert NW % WG == 0

    f32 = mybir.dt.float32
    bf16 = mybir.dt.bfloat16

    nc = bass.Bass()

    # ---- DRAM parameters (per core shapes)
    rT_d = nc.declare_dram_parameter("rT", [EMBED, T], bf16, isOutput=False)
    gT_d = nc.declare_dram_parameter("gT", [EMBED, T], bf16, isOutput=False)
    rtok_d = nc.declare_dram_parameter("rtok", [128, NW * 128], bf16, isOutput=False)
    gtok_d = nc.declare_dram_parameter("gtok", [128, NW * 128], bf16, isOutput=False)
    w1_d = [nc.declare_dram_parameter(f"w1_{b}", [EMBED, 1024], bf16, isOutput=False)
            for b in range(2)]
    w2_d = [nc.declare_dram_parameter(f"w2_{b}", [8, 128, EMBED], bf16, isOutput=False)
            for b in range(2)]
    b1_d = [nc.declare_dram_parameter(f"b1_{b}", [128, 8], f32, isOutput=False)
            for b in range(2)]
    b2_d = [nc.declare_dram_parameter(f"b2_{b}", [128, 2], f32, isOutput=False)
            for b in range(2)]
    lng_d = [nc.declare_dram_parameter(f"lng_{b}", [128, 2], f32, isOutput=False)
             for b in range(2)]
    lnb_d = [nc.declare_dram_parameter(f"lnb_{b}", [128, 2], f32, isOutput=False)
             for b in range(2)]
    onesbd_d = nc.declare_dram_parameter("onesbd", [128, 128], bf16, isOutput=False)
    ones_d = nc.declare_dram_parameter("ones", [128, 128], bf16, isOutput=False)
    x_d = [nc.declare_dram_parameter(f"x{b+1}T", [EMBED, T], bf16, isOutput=True)
           for b in range(2)]

    SCALE = 1.0 / math.sqrt(HD)

    with tile.TileContext(nc) as tc:
        with tc.tile_pool(name="persist", bufs=1) as P:
            # persistent SBUF tiles
            rT = [[P.tile([128, CH], bf16, tag=f"rT{k}_{g}", name=f"rT{k}_{g}") for g in range(NWG)]
                  for k in range(2)]
            gT = [[P.tile([128, CH], bf16, tag=f"gT{k}_{g}", name=f"gT{k}_{g}") for g in range(NWG)]
                  for k in range(2)]
            rtok = [P.tile([128, WG * 128], bf16, tag=f"rtok{g}", name=f"rtok{g}") for g in range(NWG)]
            gtok = [P.tile([128, WG * 128], bf16, tag=f"gtok{g}", name=f"gtok{g}") for g in range(NWG)]
            OT = [[P.tile([128, WG * 128], bf16, tag=f"OT{b}_{g}", name=f"OT{b}_{g}") for g in range(NWG)]
                  for b in range(2)]
            W1 = [[P.tile([128, 1024], bf16, tag=f"W1_{b}_{k}", name=f"W1_{b}_{k}") for k in range(2)]
                  for b in range(2)]
            W2 = [P.tile([128, 8 * EMBED], bf16, tag=f"W2_{b}", name=f"W2_{b}") for b in range(2)]
            B1 = [P.tile([128, 8], f32, tag=f"B1_{b}", name=f"B1_{b}") for b in range(2)]
            B2 = [P.tile([128, 2], f32, tag=f"B2_{b}", name=f"B2_{b}") for b in range(2)]
            LNG = [P.tile([128, 2], f32, tag=f"LNG_{b}", name=f"LNG_{b}") for b in range(2)]
            LNB = [P.tile([128, 2], f32, tag=f"LNB_{b}", name=f"LNB_{b}") for b in range(2)]
            OBD = P.tile([128, 128], bf16, tag="OBD", name="OBD")
            ONE = P.tile([128, 128], bf16, tag="ONE", name="ONE")
            EPS = P.tile([128, 1], f32, tag="EPS", name="EPS")
            nc.vector.memset(EPS[:, :], LN_EPS)
            C105 = P.tile([128, 1], f32, tag="C105", name="C105")
            nc.vector.memset(C105[:, :], 1.05)
            NEGG = [P.tile([128, 2], f32, tag=f"NEGG_{b}", name=f"NEGG_{b}")
                    for b in range(2)]

            # ---- input DMAs (chunked so compute can start early)
            for g in range(NWG):
                for k in range(2):
                    nc.sync.dma_start(out=rT[k][g][:, :],
                                      in_=rT_d[128 * k:128 * (k + 1), CH * g:CH * (g + 1)])
                    nc.sync.dma_start(out=gT[k][g][:, :],
                                      in_=gT_d[128 * k:128 * (k + 1), CH * g:CH * (g + 1)])
                nc.sync.dma_start(out=rtok[g][:, :],
                                  in_=rtok_d[:, WG * 128 * g: WG * 128 * (g + 1)])
                nc.sync.dma_start(out=gtok[g][:, :],
                                  in_=gtok_d[:, WG * 128 * g: WG * 128 * (g + 1)])
            nc.sync.dma_start(out=OBD[:, :], in_=onesbd_d[:, :])
            nc.sync.dma_start(out=ONE[:, :], in_=ones_d[:, :])
            for b in range(2):
                for k in range(2):
                    nc.sync.dma_start(out=W1[b][k][:, :],
                                      in_=w1_d[b][128 * k:128 * (k + 1), :])
                for k8 in range(8):
                    nc.sync.dma_start(out=W2[b][:, EMBED * k8:EMBED * (k8 + 1)],
                                      in_=w2_d[b][k8])
                nc.sync.dma_start(out=B1[b][:, :], in_=b1_d[b][:, :])
                nc.sync.dma_start(out=B2[b][:, :], in_=b2_d[b][:, :])
                nc.sync.dma_start(out=LNG[b][:, :], in_=lng_d[b][:, :])
                nc.sync.dma_start(out=LNB[b][:, :], in_=lnb_d[b][:, :])
                nc.vector.tensor_scalar_mul(NEGG[b][:, :], LNG[b][:, :], -1.0)

            # ================= phase 1: attention =================
            # Window pairs. PSUM rule: concurrent matmuls with different PE
            # row-groups but the same col-group must not share a PSUM bank.
            # ssb [128,1024] f32 (4 banks): bank i holds heads {i, i+4} (both
            # row-group 32*i): piece (h, wp, half) at partitions 64*(h//4),
            # free 256*(h%4) + 128*wp + 64*half.  Merged M=64 score matmuls.
            with tc.tile_pool(name="p1sb", bufs=3) as SB1, \
                 tc.tile_pool(name="p1ps", bufs=1, space="PSUM") as PS1:
                assert NW % 2 == 0 and WG % 2 == 0
                for p2 in range(NW // 2):
                    w0 = 2 * p2
                    g = w0 // WG
                    cw0 = w0 % WG
                    # 4 banks (512 f32 each), bank i = heads {i, i+4};
                    # valid data in first 256 f32 of each bank
                    ssb = PS1.tile([128, 2048], f32, tag="ssb", name="ssb")
                    for wp in range(2):
                        cw = cw0 + wp
                        for half in range(2):  # 0: S (lhsT=r, rhs=g), 1: S.T
                            for h in range(HEADS):
                                kc, i = h // 4, h % 4
                                qsrc = rT[kc][g] if half == 0 else gT[kc][g]
                                ksrc = gT[kc][g] if half == 0 else rT[kc][g]
                                nc.tensor.matmul(
                                    ssb[64 * kc:64 * kc + 64,
                                        512 * i + 128 * wp + 64 * half:
                                        512 * i + 128 * wp + 64 * half + 64],
                                    qsrc[32 * i:32 * i + 32,
                                         64 * cw:64 * cw + 64],
                                    ksrc[32 * i:32 * i + 32, 64 * cw:64 * cw + 64],
                                    start=True, stop=True,
                                    tile_position=(32 * i, 64 * kc),
                                )
                    # e = exp(scale * s) (bf16, compact) -- frees ssb
                    e = SB1.tile([128, 1024], bf16, tag="e", name="e")
                    ssb_v = ssb[:, :].rearrange("p (b u) -> p b u", u=512)[:, :, 0:256]
                    e_v = e[:, :].rearrange("p (b u) -> p b u", u=256)
                    nc.scalar.activation(e_v, ssb_v,
                                         mybir.ActivationFunctionType.Exp,
                                         scale=SCALE)
                    # both branches' softmax denominators: block-diag ones MM
                    rs = PS1.tile([128, 1024], f32, tag="rs", name="rs")
                    for q_ in range(2):
                        nc.tensor.matmul(rs[:, 512 * q_:512 * q_ + 512],
                                         OBD[:, :], e[:, 512 * q_:512 * q_ + 512],
                                         start=True, stop=True)
                    # rsr = exp(-ln(rs)): stays in the natural_log_exp
                    # ACT table set (no table swap), bf16 for DVE 2X mul
                    lr = SB1.tile([128, 1024], f32, tag="lr", name="lr")
                    nc.scalar.activation(lr[:, :], rs[:, :],
                                         mybir.ActivationFunctionType.Ln)
                    rsr = SB1.tile([128, 1024], bf16, tag="rsr", name="rsr")
                    nc.scalar.activation(rsr[:, :], lr[:, :],
                                         mybir.ActivationFunctionType.Exp,
                                         scale=-1.0)
                    en = SB1.tile([128, 1024], bf16, tag="en", name="en")
                    for q_ in range(2):
                        nc.vector.tensor_mul(en[:, 512 * q_:512 * q_ + 512],
                                             e[:, 512 * q_:512 * q_ + 512],
                                             rsr[:, 512 * q_:512 * q_ + 512])
                    # AV packs; o12 [128,512] (2 banks): piece (br,h,wp) at
                    # partitions 32*(h%4), free 256*(h//4) + 128*wp + 64*br
                    o12 = PS1.tile([128, 1024], f32, tag="o12", name="o12")
                    for wp in range(2):
                        cw = cw0 + wp
                        for br_ in range(2):
                            vsrc = gtok if br_ == 0 else rtok
                            for h in range(HEADS):
                                fb, i4 = h // 4, h % 4
                                nc.tensor.matmul(
                                    o12[32 * i4:32 * i4 + 32,
                                        512 * fb + 128 * wp + 64 * br_:
                                        512 * fb + 128 * wp + 64 * br_ + 64],
                                    vsrc[g][64 * fb:64 * fb + 64,
                                            128 * cw + 32 * i4:128 * cw + 32 * i4 + 32],
                                    en[64 * fb:64 * fb + 64,
                                       256 * i4 + 128 * wp + 64 * (1 - br_):
                                       256 * i4 + 128 * wp + 64 * (1 - br_) + 64],
                                    start=True, stop=True,
                                    tile_position=(64 * fb, 32 * i4),
                                )
                    # attn outputs -> persistent SBUF (bf16), one copy/branch
                    o12r = o12[:, :].rearrange(
                        "p (fb u wp br n) -> p wp fb u br n",
                        fb=2, u=2, wp=2, br=2)
                    for br_ in range(2):
                        dst = OT[br_][g][:, 128 * cw0:128 * cw0 + 256] \
                            .rearrange("p (wp fb n) -> p wp fb n", wp=2, fb=2)
                        nc.vector.tensor_copy(dst, o12r[:, :, :, 0, br_, :])

            # ================= phase 2: MLP + residual + LN =================
            # Per branch: sub-A = fc1+gelu+fc2+residual for ALL chunks (ACT
            # stays on the gelu set), then sub-B = LN stats+apply (DVE-only).
            with tc.tile_pool(name="p2sb", bufs=2) as SB2, \
                 tc.tile_pool(name="p2res", bufs=1) as SBR, \
                 tc.tile_pool(name="p2ps", bufs=2, space="PSUM") as PS2:
                for br in range(2):
                    resT = [rT, gT][br]
                    res = {}
                    for q in range(NWG):
                        otq = OT[br][q].rearrange("p (w f) -> p w f", f=128)
                        ht = SB2.tile([128, 8, CH], bf16, tag="ht", name="ht")
                        for m in range(8):
                            hp = PS2.tile([128, 512], f32, tag="hp", name="hp",
                                          bufs=3)
                            for kc in range(2):
                                nc.tensor.matmul(
                                    hp[:, :CH],
                                    W1[br][kc][:, 128 * m:128 * m + 128],
                                    otq[:, :, 64 * kc:64 * kc + 64],
                                    start=(kc == 0), stop=(kc == 1),
                                )
                            nc.scalar.activation(ht[:, m, :], hp[:, :CH],
                                                 mybir.ActivationFunctionType.Gelu,
                                                 bias=B1[br][:, m:m + 1])
                        for m2 in range(2):
                            yp = PS2.tile([128, 512], f32, tag="yp", name="yp",
                                          bufs=3)
                            for k8 in range(8):
                                nc.tensor.matmul(
                                    yp[:, :CH],
                                    W2[br][:, 256 * k8 + 128 * m2:256 * k8 + 128 * m2 + 128],
                                    ht[:, k8, :],
                                    start=(k8 == 0), stop=(k8 == 7),
                                )
                            t1 = SB2.tile([128, CH], bf16, tag=f"t1_{m2}",
                                          name=f"t1_{m2}")
                            nc.vector.scalar_tensor_tensor(
                                t1[:, :], yp[:, :CH], B2[br][:, m2:m2 + 1],
                                otq[:, :, 64 * m2:64 * m2 + 64],
                                op0=mybir.AluOpType.add, op1=mybir.AluOpType.add)
                            rr = SBR.tile([128, CH], bf16, tag=f"res_{q}_{m2}",
                                          name=f"res_{q}_{m2}")
                            nc.gpsimd.tensor_add(rr[:, :], t1[:, :],
                                                 resT[m2][q][:, :])
                            res[(q, m2)] = rr
                    # ---- sub-B: LN over all chunks (DVE-only stats)
                    for q in range(NWG):
                        sq = [SB2.tile([128, CH], bf16, tag=f"sq_{m2}",
                                       name=f"sq_{m2}") for m2 in range(2)]
                        for m2 in range(2):
                            nc.scalar.square(sq[m2][:, :], res[(q, m2)][:, :])
                        sump = PS2.tile([128, 512], f32, tag="sum", name="sum",
                                        bufs=1)
                        sqsp = PS2.tile([128, 512], f32, tag="sqs", name="sqs",
                                        bufs=1)
                        for m2 in range(2):
                            nc.tensor.matmul(sump[:, :CH], ONE[:, :],
                                             res[(q, m2)][:, :],
                                             start=(m2 == 0), stop=(m2 == 1))
                            nc.tensor.matmul(sqsp[:, :CH], ONE[:, :], sq[m2][:, :],
                                             start=(m2 == 0), stop=(m2 == 1))
                        mean = SB2.tile([128, CH], bf16, tag="mean", name="mean")
                        nc.scalar.mul(mean[:, :], sump[:, :CH], 1.0 / EMBED)
                        sq2 = SB2.tile([128, CH], bf16, tag="sq2", name="sq2")
                        nc.scalar.mul(sq2[:, :], sqsp[:, :CH], 1.0 / EMBED)
                        m2t = SB2.tile([128, CH], bf16, tag="m2t", name="m2t")
                        nc.vector.tensor_mul(m2t[:, :], mean[:, :], mean[:, :])
                        dd = SB2.tile([128, CH], bf16, tag="dd", name="dd")
                        nc.vector.tensor_sub(dd[:, :], sq2[:, :], m2t[:, :])
                        vv = SB2.tile([128, CH], bf16, tag="vv", name="vv")
                        nc.vector.tensor_scalar_add(vv[:, :], dd[:, :], EPS[:, :])
                        y0 = SB2.tile([128, CH], bf16, tag="y0", name="y0")
                        nc.vector.tensor_scalar(
                            y0[:, :], vv[:, :], -0.155, 1.05,
                            op0=mybir.AluOpType.mult, op1=mybir.AluOpType.add)
                        yc = y0
                        for it in range(1):
                            tt_ = SB2.tile([128, CH], bf16, tag=f"nt{it}",
                                           name=f"nt{it}")
                            nc.vector.tensor_mul(tt_[:, :], yc[:, :], yc[:, :])
                            ss_ = SB2.tile([128, CH], bf16, tag=f"ns{it}",
                                           name=f"ns{it}")
                            nc.vector.scalar_tensor_tensor(
                                ss_[:, :], vv[:, :], -0.5, tt_[:, :],
                                op0=mybir.AluOpType.mult, op1=mybir.AluOpType.mult)
                            yn = SB2.tile([128, CH], bf16, tag=f"ny{it}",
                                          name=f"ny{it}")
                            nc.vector.scalar_tensor_tensor(
                                yn[:, :], ss_[:, :], 1.5, yc[:, :],
                                op0=mybir.AluOpType.add, op1=mybir.AluOpType.mult)
                            yc = yn
                        rstd = yc
                        mrstd = SB2.tile([128, CH], bf16, tag="mrstd", name="mrstd")
                        nc.vector.tensor_mul(mrstd[:, :], mean[:, :], rstd[:, :])
                        for m2 in range(2):
                            ta = SB2.tile([128, CH], bf16, tag=f"ta_{m2}",
                                          name=f"ta_{m2}")
                            nc.vector.scalar_tensor_tensor(
                                ta[:, :], res[(q, m2)][:, :], LNG[br][:, m2:m2 + 1],
                                rstd[:, :],
                                op0=mybir.AluOpType.mult, op1=mybir.AluOpType.mult)
                            tb = SB2.tile([128, CH], bf16, tag=f"tb_{m2}",
                                          name=f"tb_{m2}")
                            nc.vector.tensor_scalar(
                                tb[:, :], mrstd[:, :], LNG[br][:, m2:m2 + 1],
                                LNB[br][:, m2:m2 + 1],
                                op0=mybir.AluOpType.mult,
                                op1=mybir.AluOpType.subtract)
                            y = SB2.tile([128, CH], bf16, tag=f"y_{m2}",
                                         name=f"y_{m2}")
                            nc.vector.tensor_sub(y[:, :], ta[:, :], tb[:, :])
                            nc.sync.dma_start(
                                out=x_d[br][128 * m2:128 * m2 + 128,
                                            CH * q:CH * (q + 1)],
                                in_=y[:, :])
    if split_waits:
        _split_waits(nc)
    return nc


# ---------------------------------------------------------------- host side
def _win_part(x, ws):
    B, H, W, C = x.shape
    x = x.reshape(B, H // ws, ws, W // ws, ws, C)
    return x.transpose(0, 1, 3, 2, 4, 5).reshape(-1, ws * ws, C)


def _win_unpart(wins, ws, B, H, W):
    C = wins.shape[-1]
    x = wins.reshape(B, H // ws, W // ws, ws, ws, C)
    return x.transpose(0, 1, 3, 2, 4, 5).reshape(B, H, W, C)


def make_core_inputs(r_c, g_c, weights, NW):
    """r_c, g_c: [NW, 64, C] f32 -> in_map dict for one core."""
    import ml_dtypes
    bf = ml_dtypes.bfloat16
    T = NW * N

    def featmaj(a):  # [NW, 64, C] -> [C, T]
        return np.ascontiguousarray(
            a.transpose(2, 0, 1).reshape(EMBED, T)).astype(bf)

    def tokmaj(a):  # [NW, 64, C] -> [128, NW*128]
        t = a.reshape(NW, N, 2, 128).transpose(2, 1, 0, 3)  # [fb, tok, w, cm]
        return np.ascontiguousarray(t.reshape(128, NW * 128)).astype(bf)

    m = {
        "rT": featmaj(r_c), "gT": featmaj(g_c),
        "rtok": tokmaj(r_c), "gtok": tokmaj(g_c),
        "onesbd": np.kron(np.eye(2, dtype=np.float32),
                          np.ones((64, 64), np.float32)).astype(bf),
        "ones": np.ones((128, 128), np.float32).astype(bf),
    }
    for b in range(2):
        w1, b1, w2, b2, lng, lnb = weights[b]
        m[f"w1_{b}"] = np.ascontiguousarray(w1).astype(bf)
        m[f"w2_{b}"] = np.ascontiguousarray(w2.reshape(8, 128, EMBED)).astype(bf)
        m[f"b1_{b}"] = np.ascontiguousarray(b1.reshape(8, 128).T).astype(np.float32)
        m[f"b2_{b}"] = np.ascontiguousarray(b2.reshape(2, 128).T).astype(np.float32)
        m[f"lng_{b}"] = np.ascontiguousarray(lng.reshape(2, 128).T).astype(np.float32)
        m[f"lnb_{b}"] = np.ascontiguousarray(lnb.reshape(2, 128).T).astype(np.float32)
    return m


def postprocess(res, NW):
    """res: per-core result dicts -> (x1, x2) full arrays [8, 64, 64, 256]."""
    outs = []
    for b in range(2):
        wins = np.concatenate([
            np.asarray(r[f"x{b+1}T"], np.float32)
            .reshape(EMBED, NW, N).transpose(1, 2, 0)
            for r in res], axis=0)
        outs.append(_win_unpart(wins, WS, 8, 64, 64))
    return tuple(outs)


def kernel(c1, c2, window_size, mlp1_fc1_w, mlp1_fc1_b, mlp1_fc2_w, mlp1_fc2_b,
           ln1_g, ln1_b, mlp2_fc1_w, mlp2_fc1_b, mlp2_fc2_w, mlp2_fc2_b,
           ln2_g, ln2_b):
    from concourse.bass_utils import run_bass_kernel_spmd

    ws = int(window_size)
    assert ws == WS
    c1 = np.asarray(c1, np.float32)
    c2 = np.asarray(c2, np.float32)
    B, H, W, C = c1.shape
    r = _win_part(c1, ws)
    g = _win_part(c2, ws)
    n_win = r.shape[0]
    NW = n_win // 8
    weights = [
        (np.asarray(mlp1_fc1_w, np.float32), np.asarray(mlp1_fc1_b, np.float32),
         np.asarray(mlp1_fc2_w, np.float32), np.asarray(mlp1_fc2_b, np.float32),
         np.asarray(ln1_g, np.float32), np.asarray(ln1_b, np.float32)),
        (np.asarray(mlp2_fc1_w, np.float32), np.asarray(mlp2_fc1_b, np.float32),
         np.asarray(mlp2_fc2_w, np.float32), np.asarray(mlp2_fc2_b, np.float32),
         np.asarray(ln2_g, np.float32), np.asarray(ln2_b, np.float32)),
    ]
    if NW not in _CACHE:
        _CACHE[NW] = build_nc(NW)
    nc = _CACHE[NW]
    in_maps = [make_core_inputs(r[NW * c:NW * (c + 1)], g[NW * c:NW * (c + 1)],
                                weights, NW) for c in range(8)]
    res = run_bass_kernel_spmd(nc, in_maps, list(range(8))).results
    return postprocess(res, NW)


# revision 38
# speedup vs baseline: 1.0768x; 1.0768x over previous
"""Trainium2 Bass kernel for C2AttentionBlock (windowed cross-attention, 2 branches).

Sharding: data-parallel over batch. Core b handles batch image b (64 windows of
64 tokens, C=256). All compute in bf16 matmuls / f32 accumulation.

Device layout (per core, NW windows, T = 64*NW tokens):
  rT/gT   feature-major [256, T] bf16   (2 partition tiles of 128 channels)
  rtok/gtok token-major [128, NW*128] bf16:
            partition p = 64*(c//128) + token, free = 128*w + (c%128)
  Window pairs: scores S and S.T as tile_position-packed K=32/M=64 matmuls
  into 4 PSUM banks (one per PE row-group -- concurrent matmuls that share a
  col-group but differ in row-group must not share a PSUM bank).
  Softmax for BOTH branches from one scores set: denominators via a
  block-diag-ones matmul (64-partition column sums broadcast to all 128
  partitions), then en = e * exp(-ln(rowsum)) -- ACT stays on the single
  natural_log_exp table set (table swaps cost ~1.3-2.7us each).
  AV: 8-way packed K=64 matmuls; O.T assembled feature-major in 2 banks.
  MLP: fc1 (C->4C, exact GELU, gelu table set) + fc2 feature-major;
  LayerNorm feature-major: channel sums via all-ones matmul, rsqrt via a
  DVE Newton step (keeps ACT off the exp set during the gelu phase).
  All elementwise in bf16 where possible (DVE 2X mode); outputs bf16.
"""

import math

import numpy as np

EMBED = 256
HEADS = 8
HD = 32
WS = 8
LN_EPS = 1e-5
N = 64  # tokens per window

_CACHE = {}


def _split_waits(nc, max_waits=1):
    """This walrus build only supports one sync-wait slot per instruction;
    move excess waits onto preceding same-engine NOPs."""
    import concourse.mybir as mybir
    for f in nc.m.functions:
        for blk in f.blocks:
            il = blk.instructions
            out = []
            for inst in il:
                si = inst.sync_info
                if si is not None and len(si.on_wait) > max_waits:
                    waits = list(si.on_wait)
                    for k, w in enumerate(waits[:-max_waits]):
                        nop = mybir.InstNoOp(
                            name=f"{inst.name}-ws{k}",
                            sync_info=mybir.SyncInfo(on_wait=[w], on_update=[]),
                            bass_nofuse=True,
                            engine=inst.engine,
                        )
                        out.append(nop)
                    inst.sync_info = mybir.SyncInfo(
                        on_wait=waits[-max_waits:],
                        on_update=list(si.on_update))
                out.append(inst)
            il.clear()
            il.extend(out)


def _act_recip(nc, mybir, out, in_):
    """ACT-engine reciprocal via raw InstActivation (the bass wrapper bans it
    for accuracy; ~1e-3 rel is fine for softmax denominators here)."""
    eng = nc.scalar
    ins = [eng.lower_ap(in_),
           mybir.ImmediateValue(dtype=mybir.dt.float32, value=0.0),
           mybir.ImmediateValue(dtype=mybir.dt.float32, value=1.0),
           mybir.ImmediateValue(dtype=mybir.dt.float32, value=0.0)]
    return eng.add_instruction(mybir.InstActivation(
        name=nc.get_next_instruction_name(),
        func=mybir.ActivationFunctionType.Reciprocal,
        ins=ins, outs=[eng.lower_ap(out)]))


# ---------------------------------------------------------------- builder
def build_nc(NW=64, split_waits=True):
    import concourse.bass as bass
    import concourse.mybir as mybir
    import concourse.tile as tile

    T = N * NW
    WG = min(8, NW)          # windows per group / chunk
    NWG = NW // WG           # number of groups (== token chunks)
    CH = WG * N              # tokens per chunk (512 normally)
    assert NW % WG == 0

    f32 = mybir.dt.float32
    bf16 = mybir.dt.bfloat16

    nc = bass.Bass()

    # ---- DRAM parameters (per core shapes)
    rT_d = nc.declare_dram_parameter("rT", [EMBED, T], bf16, isOutput=False)
    gT_d = nc.declare_dram_parameter("gT", [EMBED, T], bf16, isOutput=False)
    rtok_d = nc.declare_dram_parameter("rtok", [128, NW * 128], bf16, isOutput=False)
    gtok_d = nc.declare_dram_parameter("gtok", [128, NW * 128], bf16, isOutput=False)
    w1_d = [nc.declare_dram_parameter(f"w1_{b}", [EMBED, 1024], bf16, isOutput=False)
            for b in range(2)]
    w2_d = [nc.declare_dram_parameter(f"w2_{b}", [8, 128, EMBED], bf16, isOutput=False)
            for b in range(2)]
    b1_d = [nc.declare_dram_parameter(f"b1_{b}", [128, 8], f32, isOutput=False)
            for b in range(2)]
    b2_d = [nc.declare_dram_parameter(f"b2_{b}", [128, 2], f32, isOutput=False)
            for b in range(2)]
    lng_d = [nc.declare_dram_parameter(f"lng_{b}", [128, 2], f32, isOutput=False)
             for b in range(2)]
    lnb_d = [nc.declare_dram_parameter(f"lnb_{b}", [128, 2], f32, isOutput=False)
             for b in range(2)]
    onesbd_d = nc.declare_dram_parameter("onesbd", [128, 128], bf16, isOutput=False)
    ones_d = nc.declare_dram_parameter("ones", [128, 128], bf16, isOutput=False)
    x_d = [nc.declare_dram_parameter(f"x{b+1}T", [EMBED, T], bf16, isOutput=True)
           for b in range(2)]

    SCALE = 1.0 / math.sqrt(HD)

    with tile.TileContext(nc) as tc:
        with tc.tile_pool(name="persist", bufs=1) as P:
            # persistent SBUF tiles
            rT = [[P.tile([128, CH], bf16, tag=f"rT{k}_{g}", name=f"rT{k}_{g}") for g in range(NWG)]
                  for k in range(2)]
            gT = [[P.tile([128, CH], bf16, tag=f"gT{k}_{g}", name=f"gT{k}_{g}") for g in range(NWG)]
                  for k in range(2)]
            rtok = [P.tile([128, WG * 128], bf16, tag=f"rtok{g}", name=f"rtok{g}") for g in range(NWG)]
            gtok = [P.tile([128, WG * 128], bf16, tag=f"gtok{g}", name=f"gtok{g}") for g in range(NWG)]
            OT = [[P.tile([128, WG * 128], bf16, tag=f"OT{b}_{g}", name=f"OT{b}_{g}") for g in range(NWG)]
                  for b in range(2)]
            W1 = [[P.tile([128, 1024], bf16, tag=f"W1_{b}_{k}", name=f"W1_{b}_{k}") for k in range(2)]
                  for b in range(2)]
            W2 = [P.tile([128, 8 * EMBED], bf16, tag=f"W2_{b}", name=f"W2_{b}") for b in range(2)]
            B1 = [P.tile([128, 8], f32, tag=f"B1_{b}", name=f"B1_{b}") for b in range(2)]
            B2 = [P.tile([128, 2], f32, tag=f"B2_{b}", name=f"B2_{b}") for b in range(2)]
            LNG = [P.tile([128, 2], f32, tag=f"LNG_{b}", name=f"LNG_{b}") for b in range(2)]
            LNB = [P.tile([128, 2], f32, tag=f"LNB_{b}", name=f"LNB_{b}") for b in range(2)]
            OBD = P.tile([128, 128], bf16, tag="OBD", name="OBD")
            ONE = P.tile([128, 128], bf16, tag="ONE", name="ONE")
            EPS = P.tile([128, 1], f32, tag="EPS", name="EPS")
            nc.vector.memset(EPS[:, :], LN_EPS)
            C105 = P.tile([128, 1], f32, tag="C105", name="C105")
            nc.vector.memset(C105[:, :], 1.05)
            NEGG = [P.tile([128, 2], f32, tag=f"NEGG_{b}", name=f"NEGG_{b}")
                    for b in range(2)]

            # ---- input DMAs (chunked so compute can start early)
            for g in range(NWG):
                for k in range(2):
                    nc.sync.dma_start(out=rT[k][g][:, :],
                                      in_=rT_d[128 * k:128 * (k + 1), CH * g:CH * (g + 1)])
                    nc.sync.dma_start(out=gT[k][g][:, :],
                                      in_=gT_d[128 * k:128 * (k + 1), CH * g:CH * (g + 1)])
                nc.sync.dma_start(out=rtok[g][:, :],
                                  in_=rtok_d[:, WG * 128 * g: WG * 128 * (g + 1)])
                nc.sync.dma_start(out=gtok[g][:, :],
                                  in_=gtok_d[:, WG * 128 * g: WG * 128 * (g + 1)])
            nc.sync.dma_start(out=OBD[:, :], in_=onesbd_d[:, :])
            nc.sync.dma_start(out=ONE[:, :], in_=ones_d[:, :])
            for b in range(2):
                for k in range(2):
                    nc.sync.dma_start(out=W1[b][k][:, :],
                                      in_=w1_d[b][128 * k:128 * (k + 1), :])
                for k8 in range(8):
                    nc.sync.dma_start(out=W2[b][:, EMBED * k8:EMBED * (k8 + 1)],
                                      in_=w2_d[b][k8])
                nc.sync.dma_start(out=B1[b][:, :], in_=b1_d[b][:, :])
                nc.sync.dma_start(out=B2[b][:, :], in_=b2_d[b][:, :])
                nc.sync.dma_start(out=LNG[b][:, :], in_=lng_d[b][:, :])
                nc.sync.dma_start(out=LNB[b][:, :], in_=lnb_d[b][:, :])
                nc.vector.tensor_scalar_mul(NEGG[b][:, :], LNG[b][:, :], -1.0)

            # ================= phase 1: attention =================
            # Window pairs. PSUM rule: concurrent matmuls with different PE
            # row-groups but the same col-group must not share a PSUM bank.
            # ssb [128,1024] f32 (4 banks): bank i holds heads {i, i+4} (both
            # row-group 32*i): piece (h, wp, half) at partitions 64*(h//4),
            # free 256*(h%4) + 128*wp + 64*half.  Merged M=64 score matmuls.
            with tc.tile_pool(name="p1sb", bufs=3) as SB1, \
                 tc.tile_pool(name="p1ps", bufs=1, space="PSUM") as PS1:
                assert NW % 2 == 0 and WG % 2 == 0
                for p2 in range(NW // 2):
                    w0 = 2 * p2
                    g = w0 // WG
                    cw0 = w0 % WG
                    # 4 banks (512 f32 each), bank i = heads {i, i+4};
                    # valid data in first 256 f32 of each bank
                    ssb = PS1.tile([128, 2048], f32, tag="ssb", name="ssb")
                    for wp in range(2):
                        cw = cw0 + wp
                        for half in range(2):  # 0: S (lhsT=r, rhs=g), 1: S.T
                            for h in range(HEADS):
                                kc, i = h // 4, h % 4
                                qsrc = rT[kc][g] if half == 0 else gT[kc][g]
                                ksrc = gT[kc][g] if half == 0 else rT[kc][g]
                                nc.tensor.matmul(
                                    ssb[64 * kc:64 * kc + 64,
                                        512 * i + 128 * wp + 64 * half:
                                        512 * i + 128 * wp + 64 * half + 64],
                                    qsrc[32 * i:32 * i + 32,
                                         64 * cw:64 * cw + 64],
                                    ksrc[32 * i:32 * i + 32, 64 * cw:64 * cw + 64],
                                    start=True, stop=True,
                                    tile_position=(32 * i, 64 * kc),
                                )
                    # e = exp(scale * s) (bf16, compact) -- frees ssb
                    e = SB1.tile([128, 1024], bf16, tag="e", name="e")
                    ssb_v = ssb[:, :].rearrange("p (b u) -> p b u", u=512)[:, :, 0:256]
                    e_v = e[:, :].rearrange("p (b u) -> p b u", u=256)
                    nc.scalar.activation(e_v, ssb_v,
                                         mybir.ActivationFunctionType.Exp,
                                         scale=SCALE)
                    # both branches' softmax denominators: block-diag ones MM
                    rs = PS1.tile([128, 1024], f32, tag="rs", name="rs")
                    for q_ in range(2):
                        nc.tensor.matmul(rs[:, 512 * q_:512 * q_ + 512],
                                         OBD[:, :], e[:, 512 * q_:512 * q_ + 512],
                                         start=True, stop=True)
                    # rsr = exp(-ln(rs)): stays in the natural_log_exp
                    # ACT table set (no table swap), bf16 for DVE 2X mul
                    lr = SB1.tile([128, 1024], f32, tag="lr", name="lr")
                    nc.scalar.activation(lr[:, :], rs[:, :],
                                         mybir.ActivationFunctionType.Ln)
                    rsr = SB1.tile([128, 1024], bf16, tag="rsr", name="rsr")
                    nc.scalar.activation(rsr[:, :], lr[:, :],
                                         mybir.ActivationFunctionType.Exp,
                                         scale=-1.0)
                    en = SB1.tile([128, 1024], bf16, tag="en", name="en")
                    for q_ in range(2):
                        nc.vector.tensor_mul(en[:, 512 * q_:512 * q_ + 512],
                                             e[:, 512 * q_:512 * q_ + 512],
                                             rsr[:, 512 * q_:512 * q_ + 512])
                    # AV packs; o12 [128,512] (2 banks): piece (br,h,wp) at
                    # partitions 32*(h%4), free 256*(h//4) + 128*wp + 64*br
                    o12 = PS1.tile([128, 1024], f32, tag="o12", name="o12")
                    for wp in range(2):
                        cw = cw0 + wp
                        for br_ in range(2):
                            vsrc = gtok if br_ == 0 else rtok
                            for h in range(HEADS):
                                fb, i4 = h // 4, h % 4
                                nc.tensor.matmul(
                                    o12[32 * i4:32 * i4 + 32,
                                        512 * fb + 128 * wp + 64 * br_:
                                        512 * fb + 128 * wp + 64 * br_ + 64],
                                    vsrc[g][64 * fb:64 * fb + 64,
                                            128 * cw + 32 * i4:128 * cw + 32 * i4 + 32],
                                    en[64 * fb:64 * fb + 64,
                                       256 * i4 + 128 * wp + 64 * (1 - br_):
                                       256 * i4 + 128 * wp + 64 * (1 - br_) + 64],
                                    start=True, stop=True,
                                    tile_position=(64 * fb, 32 * i4),
                                )
                    # attn outputs -> persistent SBUF (bf16), one copy/branch
                    o12r = o12[:, :].rearrange(
                        "p (fb u wp br n) -> p wp fb u br n",
                        fb=2, u=2, wp=2, br=2)
                    for br_ in range(2):
                        dst = OT[br_][g][:, 128 * cw0:128 * cw0 + 256] \
                            .rearrange("p (wp fb n) -> p wp fb n", wp=2, fb=2)
                        nc.vector.tensor_copy(dst, o12r[:, :, :, 0, br_, :])

            # ================= phase 2: MLP + residual + LN =================
            # Per branch: sub-A = fc1+gelu+fc2+residual for ALL chunks (ACT
            # stays on the gelu set), then sub-B = LN stats+apply (DVE-only).
            with tc.tile_pool(name="p2sb", bufs=2) as SB2, \
                 tc.tile_pool(name="p2res", bufs=1) as SBR, \
                 tc.tile_pool(name="p2ps", bufs=2, space="PSUM") as PS2:
                for br in range(2):
                    resT = [rT, gT][br]
                    res = {}
                    for q in range(NWG):
                        otq = OT[br][q].rearrange("p (w f) -> p w f", f=128)
                        ht = SB2.tile([128, 8, CH], bf16, tag="ht", name="ht")
                        for m in range(8):
                            hp = PS2.tile([128, 512], f32, tag="hp", name="hp")
                            for kc in range(2):
                                nc.tensor.matmul(
                                    hp[:, :CH],
                                    W1[br][kc][:, 128 * m:128 * m + 128],
                                    otq[:, :, 64 * kc:64 * kc + 64],
                                    start=(kc == 0), stop=(kc == 1),
                                )
                            nc.scalar.activation(ht[:, m, :], hp[:, :CH],
                                                 mybir.ActivationFunctionType.Gelu,
                                                 bias=B1[br][:, m:m + 1])
                        for m2 in range(2):
                            yp = PS2.tile([128, 512], f32, tag="yp", name="yp")
                            for k8 in range(8):
                                nc.tensor.matmul(
                                    yp[:, :CH],
                                    W2[br][:, 256 * k8 + 128 * m2:256 * k8 + 128 * m2 + 128],
                                    ht[:, k8, :],
                                    start=(k8 == 0), stop=(k8 == 7),
                                )
                            t1 = SB2.tile([128, CH], bf16, tag=f"t1_{m2}",
                                          name=f"t1_{m2}")
                            nc.vector.scalar_tensor_tensor(
                                t1[:, :], yp[:, :CH], B2[br][:, m2:m2 + 1],
                                otq[:, :, 64 * m2:64 * m2 + 64],
                                op0=mybir.AluOpType.add, op1=mybir.AluOpType.add)
                            rr = SBR.tile([128, CH], bf16, tag=f"res_{q}_{m2}",
                                          name=f"res_{q}_{m2}")
                            nc.gpsimd.tensor_add(rr[:, :], t1[:, :],
                                                 resT[m2][q][:, :])
                            res[(q, m2)] = rr
                    # ---- sub-B: LN over all chunks (DVE-only stats)
                    for q in range(NWG):
                        sq = [SB2.tile([128, CH], bf16, tag=f"sq_{m2}",
                                       name=f"sq_{m2}") for m2 in range(2)]
                        for m2 in range(2):
                            nc.scalar.square(sq[m2][:, :], res[(q, m2)][:, :])
                        sump = PS2.tile([128, 512], f32, tag="sum", name="sum")
                        sqsp = PS2.tile([128, 512], f32, tag="sqs", name="sqs")
                        for m2 in range(2):
                            nc.tensor.matmul(sump[:, :CH], ONE[:, :],
                                             res[(q, m2)][:, :],
                                             start=(m2 == 0), stop=(m2 == 1))
                            nc.tensor.matmul(sqsp[:, :CH], ONE[:, :], sq[m2][:, :],
                                             start=(m2 == 0), stop=(m2 == 1))
                        mean = SB2.tile([128, CH], bf16, tag="mean", name="mean")
                        nc.scalar.mul(mean[:, :], sump[:, :CH], 1.0 / EMBED)
                        sq2 = SB2.tile([128, CH], bf16, tag="sq2", name="sq2")
                        nc.scalar.mul(sq2[:, :], sqsp[:, :CH], 1.0 / EMBED)
                        m2t = SB2.tile([128, CH], bf16, tag="m2t", name="m2t")
                        nc.vector.tensor_mul(m2t[:, :], mean[:, :], mean[:, :])
                        dd = SB2.tile([128, CH], bf16, tag="dd", name="dd")
                        nc.vector.tensor_sub(dd[:, :], sq2[:, :], m2t[:, :])
                        vv = SB2.tile([128, CH], bf16, tag="vv", name="vv")
                        nc.vector.tensor_scalar_add(vv[:, :], dd[:, :], EPS[:, :])
                        y0 = SB2.tile([128, CH], bf16, tag="y0", name="y0")
                        nc.vector.tensor_scalar(
                            y0[:, :], vv[:, :], -0.155, 1.05,
                            op0=mybir.AluOpType.mult, op1=mybir.AluOpType.add)
                        yc = y0
                        for it in range(1):
                            tt_ = SB2.tile([128, CH], bf16, tag=f"nt{it}",
                                           name=f"nt{it}")
                            nc.vector.tensor_mul(tt_[:, :], yc[:, :], yc[:, :])
                            ss_ = SB2.tile([128, CH], bf16, tag=f"ns{it}",
                                           name=f"ns{it}")
                            nc.vector.scalar_tensor_tensor(
                                ss_[:, :], vv[:, :], -0.5, tt_[:, :],
                                op0=mybir.AluOpType.mult, op1=mybir.AluOpType.mult)
                            yn = SB2.tile([128, CH], bf16, tag=f"ny{it}",
                                          name=f"ny{it}")
                            nc.vector.scalar_tensor_tensor(
                                yn[:, :], ss_[:, :], 1.5, yc[:, :],
                                op0=mybir.AluOpType.add, op1=mybir.AluOpType.mult)
                            yc = yn
                        rstd = yc
                        mrstd = SB2.tile([128, CH], bf16, tag="mrstd", name="mrstd")
                        nc.vector.tensor_mul(mrstd[:, :], mean[:, :], rstd[:, :])
                        for m2 in range(2):
                            ta = SB2.tile([128, CH], bf16, tag=f"ta_{m2}",
                                          name=f"ta_{m2}")
                            nc.vector.scalar_tensor_tensor(
                                ta[:, :], res[(q, m2)][:, :], LNG[br][:, m2:m2 + 1],
                                rstd[:, :],
                                op0=mybir.AluOpType.mult, op1=mybir.AluOpType.mult)
                            tb = SB2.tile([128, CH], bf16, tag=f"tb_{m2}",
                                          name=f"tb_{m2}")
                            nc.vector.tensor_scalar(
                                tb[:, :], mrstd[:, :], LNG[br][:, m2:m2 + 1],
                                LNB[br][:, m2:m2 + 1],
                                op0=mybir.AluOpType.mult,
                                op1=mybir.AluOpType.subtract)
                            y = SB2.tile([128, CH], bf16, tag=f"y_{m2}",
                                         name=f"y_{m2}")
                            nc.vector.tensor_sub(y[:, :], ta[:, :], tb[:, :])
                            nc.sync.dma_start(
                                out=x_d[br][128 * m2:128 * m2 + 128,
                                            CH * q:CH * (q + 1)],
                                in_=y[:, :])
    if split_waits:
        _split_waits(nc)
    return nc


# ---------------------------------------------------------------- host side
def _win_part(x, ws):
    B, H, W, C = x.shape
    x = x.reshape(B, H // ws, ws, W // ws, ws, C)
    return x.transpose(0, 1, 3, 2, 4, 5).reshape(-1, ws * ws, C)


def _win_unpart(wins, ws, B, H, W):
    C = wins.shape[-1]
    x = wins.reshape(B, H // ws, W // ws, ws, ws, C)
    return x.transpose(0, 1, 3, 2, 4, 5).reshape(B, H, W, C)


def make_core_inputs(r_c, g_c, weights, NW):
    """r_c, g_c: [NW, 64, C] f32 -> in_map dict for one core."""
    import ml_dtypes
    bf = ml_dtypes.bfloat16
    T = NW * N

    def featmaj(a):  # [NW, 64, C] -> [C, T]
        return np.ascontiguousarray(
            a.transpose(2, 0, 1).reshape(EMBED, T)).astype(bf)

    def tokmaj(a):  # [NW, 64, C] -> [128, NW*128]
        t = a.reshape(NW, N, 2, 128).transpose(2, 1, 0, 3)  # [fb, tok, w, cm]
        return np.ascontiguousarray(t.reshape(128, NW * 128)).astype(bf)

    m = {
        "rT": featmaj(r_c), "gT": featmaj(g_c),
        "rtok": tokmaj(r_c), "gtok": tokmaj(g_c),
        "onesbd": np.kron(np.eye(2, dtype=np.float32),
                          np.ones((64, 64), np.float32)).astype(bf),
        "ones": np.ones((128, 128), np.float32).astype(bf),
    }
    for b in range(2):
        w1, b1, w2, b2, lng, lnb = weights[b]
        m[f"w1_{b}"] = np.ascontiguousarray(w1).astype(bf)
        m[f"w2_{b}"] = np.ascontiguousarray(w2.reshape(8, 128, EMBED)).astype(bf)
        m[f"b1_{b}"] = np.ascontiguousarray(b1.reshape(8, 128).T).astype(np.float32)
        m[f"b2_{b}"] = np.ascontiguousarray(b2.reshape(2, 128).T).astype(np.float32)
        m[f"lng_{b}"] = np.ascontiguousarray(lng.reshape(2, 128).T).astype(np.float32)
        m[f"lnb_{b}"] = np.ascontiguousarray(lnb.reshape(2, 128).T).astype(np.float32)
    return m


def postprocess(res, NW):
    """res: per-core result dicts -> (x1, x2) full arrays [8, 64, 64, 256]."""
    outs = []
    for b in range(2):
        wins = np.concatenate([
            np.asarray(r[f"x{b+1}T"], np.float32)
            .reshape(EMBED, NW, N).transpose(1, 2, 0)
            for r in res], axis=0)
        outs.append(_win_unpart(wins, WS, 8, 64, 64))
    return tuple(outs)


def kernel(c1, c2, window_size, mlp1_fc1_w, mlp1_fc1_b, mlp1_fc2_w, mlp1_fc2_b,
           ln1_g, ln1_b, mlp2_fc1_w, mlp2_fc1_b, mlp2_fc2_w, mlp2_fc2_b,
           ln2_g, ln2_b):
    from concourse.bass_utils import run_bass_kernel_spmd

    ws = int(window_size)
    assert ws == WS
    c1 = np.asarray(c1, np.float32)
    c2 = np.asarray(c2, np.float32)
    B, H, W, C = c1.shape
    r = _win_part(c1, ws)
    g = _win_part(c2, ws)
    n_win = r.shape[0]
    NW = n_win // 8
    weights = [
        (np.asarray(mlp1_fc1_w, np.float32), np.asarray(mlp1_fc1_b, np.float32),
         np.asarray(mlp1_fc2_w, np.float32), np.asarray(mlp1_fc2_b, np.float32),
         np.asarray(ln1_g, np.float32), np.asarray(ln1_b, np.float32)),
        (np.asarray(mlp2_fc1_w, np.float32), np.asarray(mlp2_fc1_b, np.float32),
         np.asarray(mlp2_fc2_w, np.float32), np.asarray(mlp2_fc2_b, np.float32),
         np.asarray(ln2_g, np.float32), np.asarray(ln2_b, np.float32)),
    ]
    if NW not in _CACHE:
        _CACHE[NW] = build_nc(NW)
    nc = _CACHE[NW]
    in_maps = [make_core_inputs(r[NW * c:NW * (c + 1)], g[NW * c:NW * (c + 1)],
                                weights, NW) for c in range(8)]
    res = run_bass_kernel_spmd(nc, in_maps, list(range(8))).results
    return postprocess(res, NW)


# revision 39
# speedup vs baseline: 1.0790x; 1.0021x over previous
"""Trainium2 Bass kernel for C2AttentionBlock (windowed cross-attention, 2 branches).

Sharding: data-parallel over batch. Core b handles batch image b (64 windows of
64 tokens, C=256). All compute in bf16 matmuls / f32 accumulation.

Device layout (per core, NW windows, T = 64*NW tokens):
  rT/gT   feature-major [256, T] bf16   (2 partition tiles of 128 channels)
  rtok/gtok token-major [128, NW*128] bf16:
            partition p = 64*(c//128) + token, free = 128*w + (c%128)
  Window pairs: scores S and S.T as tile_position-packed K=32/M=64 matmuls
  into 4 PSUM banks (one per PE row-group -- concurrent matmuls that share a
  col-group but differ in row-group must not share a PSUM bank).
  Softmax for BOTH branches from one scores set: denominators via a
  block-diag-ones matmul (64-partition column sums broadcast to all 128
  partitions), then en = e * exp(-ln(rowsum)) -- ACT stays on the single
  natural_log_exp table set (table swaps cost ~1.3-2.7us each).
  AV: 8-way packed K=64 matmuls; O.T assembled feature-major in 2 banks.
  MLP: fc1 (C->4C, exact GELU, gelu table set) + fc2 feature-major;
  LayerNorm feature-major: channel sums via all-ones matmul, rsqrt via a
  DVE Newton step (keeps ACT off the exp set during the gelu phase).
  All elementwise in bf16 where possible (DVE 2X mode); outputs bf16.
"""

import math

import numpy as np

EMBED = 256
HEADS = 8
HD = 32
WS = 8
LN_EPS = 1e-5
N = 64  # tokens per window

_CACHE = {}


def _split_waits(nc, max_waits=1):
    """This walrus build only supports one sync-wait slot per instruction;
    move excess waits onto preceding same-engine NOPs."""
    import concourse.mybir as mybir
    for f in nc.m.functions:
        for blk in f.blocks:
            il = blk.instructions
            out = []
            for inst in il:
                si = inst.sync_info
                if si is not None and len(si.on_wait) > max_waits:
                    waits = list(si.on_wait)
                    for k, w in enumerate(waits[:-max_waits]):
                        nop = mybir.InstNoOp(
                            name=f"{inst.name}-ws{k}",
                            sync_info=mybir.SyncInfo(on_wait=[w], on_update=[]),
                            bass_nofuse=True,
                            engine=inst.engine,
                        )
                        out.append(nop)
                    inst.sync_info = mybir.SyncInfo(
                        on_wait=waits[-max_waits:],
                        on_update=list(si.on_update))
                out.append(inst)
            il.clear()
            il.extend(out)


def _act_recip(nc, mybir, out, in_):
    """ACT-engine reciprocal via raw InstActivation (the bass wrapper bans it
    for accuracy; ~1e-3 rel is fine for softmax denominators here)."""
    eng = nc.scalar
    ins = [eng.lower_ap(in_),
           mybir.ImmediateValue(dtype=mybir.dt.float32, value=0.0),
           mybir.ImmediateValue(dtype=mybir.dt.float32, value=1.0),
           mybir.ImmediateValue(dtype=mybir.dt.float32, value=0.0)]
    return eng.add_instruction(mybir.InstActivation(
        name=nc.get_next_instruction_name(),
        func=mybir.ActivationFunctionType.Reciprocal,
        ins=ins, outs=[eng.lower_ap(out)]))


# ---------------------------------------------------------------- builder
def build_nc(NW=64, split_waits=True):
    import concourse.bass as bass
    import concourse.mybir as mybir
    import concourse.tile as tile

    T = N * NW
    WG = min(8, NW)          # windows per group / chunk
    NWG = NW // WG           # number of groups (== token chunks)
    CH = WG * N              # tokens per chunk (512 normally)
    assert NW % WG == 0

    f32 = mybir.dt.float32
    bf16 = mybir.dt.bfloat16

    nc = bass.Bass()

    # ---- DRAM parameters (per core shapes)
    rT_d = nc.declare_dram_parameter("rT", [EMBED, T], bf16, isOutput=False)
    gT_d = nc.declare_dram_parameter("gT", [EMBED, T], bf16, isOutput=False)
    rtok_d = nc.declare_dram_parameter("rtok", [128, NW * 128], bf16, isOutput=False)
    gtok_d = nc.declare_dram_parameter("gtok", [128, NW * 128], bf16, isOutput=False)
    w1_d = [nc.declare_dram_parameter(f"w1_{b}", [EMBED, 1024], bf16, isOutput=False)
            for b in range(2)]
    w2_d = [nc.declare_dram_parameter(f"w2_{b}", [8, 128, EMBED], bf16, isOutput=False)
            for b in range(2)]
    b1_d = [nc.declare_dram_parameter(f"b1_{b}", [128, 8], f32, isOutput=False)
            for b in range(2)]
    b2_d = [nc.declare_dram_parameter(f"b2_{b}", [128, 2], f32, isOutput=False)
            for b in range(2)]
    lng_d = [nc.declare_dram_parameter(f"lng_{b}", [128, 2], f32, isOutput=False)
             for b in range(2)]
    lnb_d = [nc.declare_dram_parameter(f"lnb_{b}", [128, 2], f32, isOutput=False)
             for b in range(2)]
    onesbd_d = nc.declare_dram_parameter("onesbd", [128, 128], bf16, isOutput=False)
    ones_d = nc.declare_dram_parameter("ones", [128, 128], bf16, isOutput=False)
    x_d = [nc.declare_dram_parameter(f"x{b+1}T", [EMBED, T], bf16, isOutput=True)
           for b in range(2)]

    SCALE = 1.0 / math.sqrt(HD)

    with tile.TileContext(nc) as tc:
        with tc.tile_pool(name="persist", bufs=1) as P:
            # persistent SBUF tiles
            rT = [[P.tile([128, CH], bf16, tag=f"rT{k}_{g}", name=f"rT{k}_{g}") for g in range(NWG)]
                  for k in range(2)]
            gT = [[P.tile([128, CH], bf16, tag=f"gT{k}_{g}", name=f"gT{k}_{g}") for g in range(NWG)]
                  for k in range(2)]
            rtok = [P.tile([128, WG * 128], bf16, tag=f"rtok{g}", name=f"rtok{g}") for g in range(NWG)]
            gtok = [P.tile([128, WG * 128], bf16, tag=f"gtok{g}", name=f"gtok{g}") for g in range(NWG)]
            OT = [[P.tile([128, WG * 128], bf16, tag=f"OT{b}_{g}", name=f"OT{b}_{g}") for g in range(NWG)]
                  for b in range(2)]
            W1 = [[P.tile([128, 1024], bf16, tag=f"W1_{b}_{k}", name=f"W1_{b}_{k}") for k in range(2)]
                  for b in range(2)]
            W2 = [P.tile([128, 8 * EMBED], bf16, tag=f"W2_{b}", name=f"W2_{b}") for b in range(2)]
            B1 = [P.tile([128, 8], f32, tag=f"B1_{b}", name=f"B1_{b}") for b in range(2)]
            B2 = [P.tile([128, 2], f32, tag=f"B2_{b}", name=f"B2_{b}") for b in range(2)]
            LNG = [P.tile([128, 2], f32, tag=f"LNG_{b}", name=f"LNG_{b}") for b in range(2)]
            LNB = [P.tile([128, 2], f32, tag=f"LNB_{b}", name=f"LNB_{b}") for b in range(2)]
            OBD = P.tile([128, 128], bf16, tag="OBD", name="OBD")
            ONE = P.tile([128, 128], bf16, tag="ONE", name="ONE")
            EPS = P.tile([128, 1], f32, tag="EPS", name="EPS")
            nc.vector.memset(EPS[:, :], LN_EPS)
            C105 = P.tile([128, 1], f32, tag="C105", name="C105")
            nc.vector.memset(C105[:, :], 1.05)
            NEGG = [P.tile([128, 2], f32, tag=f"NEGG_{b}", name=f"NEGG_{b}")
                    for b in range(2)]

            # ---- input DMAs (chunked so compute can start early)
            for g in range(NWG):
                for k in range(2):
                    nc.sync.dma_start(out=rT[k][g][:, :],
                                      in_=rT_d[128 * k:128 * (k + 1), CH * g:CH * (g + 1)])
                    nc.sync.dma_start(out=gT[k][g][:, :],
                                      in_=gT_d[128 * k:128 * (k + 1), CH * g:CH * (g + 1)])
                nc.sync.dma_start(out=rtok[g][:, :],
                                  in_=rtok_d[:, WG * 128 * g: WG * 128 * (g + 1)])
                nc.sync.dma_start(out=gtok[g][:, :],
                                  in_=gtok_d[:, WG * 128 * g: WG * 128 * (g + 1)])
            nc.sync.dma_start(out=OBD[:, :], in_=onesbd_d[:, :])
            nc.sync.dma_start(out=ONE[:, :], in_=ones_d[:, :])
            for b in range(2):
                for k in range(2):
                    nc.sync.dma_start(out=W1[b][k][:, :],
                                      in_=w1_d[b][128 * k:128 * (k + 1), :])
                for k8 in range(8):
                    nc.sync.dma_start(out=W2[b][:, EMBED * k8:EMBED * (k8 + 1)],
                                      in_=w2_d[b][k8])
                nc.sync.dma_start(out=B1[b][:, :], in_=b1_d[b][:, :])
                nc.sync.dma_start(out=B2[b][:, :], in_=b2_d[b][:, :])
                nc.sync.dma_start(out=LNG[b][:, :], in_=lng_d[b][:, :])
                nc.sync.dma_start(out=LNB[b][:, :], in_=lnb_d[b][:, :])
                nc.vector.tensor_scalar_mul(NEGG[b][:, :], LNG[b][:, :], -1.0)

            # ================= phase 1: attention =================
            # Window pairs. PSUM rule: concurrent matmuls with different PE
            # row-groups but the same col-group must not share a PSUM bank.
            # ssb [128,1024] f32 (4 banks): bank i holds heads {i, i+4} (both
            # row-group 32*i): piece (h, wp, half) at partitions 64*(h//4),
            # free 256*(h%4) + 128*wp + 64*half.  Merged M=64 score matmuls.
            with tc.tile_pool(name="p1sb", bufs=4) as SB1, \
                 tc.tile_pool(name="p1ps", bufs=1, space="PSUM") as PS1:
                assert NW % 2 == 0 and WG % 2 == 0
                for p2 in range(NW // 2):
                    w0 = 2 * p2
                    g = w0 // WG
                    cw0 = w0 % WG
                    # 4 banks (512 f32 each), bank i = heads {i, i+4};
                    # valid data in first 256 f32 of each bank
                    ssb = PS1.tile([128, 2048], f32, tag="ssb", name="ssb")
                    for wp in range(2):
                        cw = cw0 + wp
                        for half in range(2):  # 0: S (lhsT=r, rhs=g), 1: S.T
                            for h in range(HEADS):
                                kc, i = h // 4, h % 4
                                qsrc = rT[kc][g] if half == 0 else gT[kc][g]
                                ksrc = gT[kc][g] if half == 0 else rT[kc][g]
                                nc.tensor.matmul(
                                    ssb[64 * kc:64 * kc + 64,
                                        512 * i + 128 * wp + 64 * half:
                                        512 * i + 128 * wp + 64 * half + 64],
                                    qsrc[32 * i:32 * i + 32,
                                         64 * cw:64 * cw + 64],
                                    ksrc[32 * i:32 * i + 32, 64 * cw:64 * cw + 64],
                                    start=True, stop=True,
                                    tile_position=(32 * i, 64 * kc),
                                )
                    # e = exp(scale * s) (bf16, compact) -- frees ssb
                    e = SB1.tile([128, 1024], bf16, tag="e", name="e")
                    ssb_v = ssb[:, :].rearrange("p (b u) -> p b u", u=512)[:, :, 0:256]
                    e_v = e[:, :].rearrange("p (b u) -> p b u", u=256)
                    nc.scalar.activation(e_v, ssb_v,
                                         mybir.ActivationFunctionType.Exp,
                                         scale=SCALE)
                    # both branches' softmax denominators: block-diag ones MM
                    rs = PS1.tile([128, 1024], f32, tag="rs", name="rs")
                    for q_ in range(2):
                        nc.tensor.matmul(rs[:, 512 * q_:512 * q_ + 512],
                                         OBD[:, :], e[:, 512 * q_:512 * q_ + 512],
                                         start=True, stop=True)
                    # rsr = exp(-ln(rs)): stays in the natural_log_exp
                    # ACT table set (no table swap), bf16 for DVE 2X mul
                    lr = SB1.tile([128, 1024], f32, tag="lr", name="lr")
                    nc.scalar.activation(lr[:, :], rs[:, :],
                                         mybir.ActivationFunctionType.Ln)
                    rsr = SB1.tile([128, 1024], bf16, tag="rsr", name="rsr")
                    nc.scalar.activation(rsr[:, :], lr[:, :],
                                         mybir.ActivationFunctionType.Exp,
                                         scale=-1.0)
                    en = SB1.tile([128, 1024], bf16, tag="en", name="en")
                    nc.vector.tensor_mul(en[:, :], e[:, :], rsr[:, :])
                    # AV packs; o12 [128,512] (2 banks): piece (br,h,wp) at
                    # partitions 32*(h%4), free 256*(h//4) + 128*wp + 64*br
                    o12 = PS1.tile([128, 1024], f32, tag="o12", name="o12")
                    for wp in range(2):
                        cw = cw0 + wp
                        for br_ in range(2):
                            vsrc = gtok if br_ == 0 else rtok
                            for h in range(HEADS):
                                fb, i4 = h // 4, h % 4
                                nc.tensor.matmul(
                                    o12[32 * i4:32 * i4 + 32,
                                        512 * fb + 128 * wp + 64 * br_:
                                        512 * fb + 128 * wp + 64 * br_ + 64],
                                    vsrc[g][64 * fb:64 * fb + 64,
                                            128 * cw + 32 * i4:128 * cw + 32 * i4 + 32],
                                    en[64 * fb:64 * fb + 64,
                                       256 * i4 + 128 * wp + 64 * (1 - br_):
                                       256 * i4 + 128 * wp + 64 * (1 - br_) + 64],
                                    start=True, stop=True,
                                    tile_position=(64 * fb, 32 * i4),
                                )
                    # attn outputs -> persistent SBUF (bf16), one copy/branch
                    o12r = o12[:, :].rearrange(
                        "p (fb u wp br n) -> p wp fb u br n",
                        fb=2, u=2, wp=2, br=2)
                    for br_ in range(2):
                        dst = OT[br_][g][:, 128 * cw0:128 * cw0 + 256] \
                            .rearrange("p (wp fb n) -> p wp fb n", wp=2, fb=2)
                        nc.vector.tensor_copy(dst, o12r[:, :, :, 0, br_, :])

            # ================= phase 2: MLP + residual + LN =================
            # Per branch: sub-A = fc1+gelu+fc2+residual for ALL chunks (ACT
            # stays on the gelu set), then sub-B = LN stats+apply (DVE-only).
            with tc.tile_pool(name="p2sb", bufs=2) as SB2, \
                 tc.tile_pool(name="p2res", bufs=1) as SBR, \
                 tc.tile_pool(name="p2ps", bufs=2, space="PSUM") as PS2:
                for br in range(2):
                    resT = [rT, gT][br]
                    res = {}
                    for q in range(NWG):
                        otq = OT[br][q].rearrange("p (w f) -> p w f", f=128)
                        ht = SB2.tile([128, 8, CH], bf16, tag="ht", name="ht", bufs=3)
                        for m in range(8):
                            hp = PS2.tile([128, 512], f32, tag="hp", name="hp")
                            for kc in range(2):
                                nc.tensor.matmul(
                                    hp[:, :CH],
                                    W1[br][kc][:, 128 * m:128 * m + 128],
                                    otq[:, :, 64 * kc:64 * kc + 64],
                                    start=(kc == 0), stop=(kc == 1),
                                )
                            nc.scalar.activation(ht[:, m, :], hp[:, :CH],
                                                 mybir.ActivationFunctionType.Gelu,
                                                 bias=B1[br][:, m:m + 1])
                        for m2 in range(2):
                            yp = PS2.tile([128, 512], f32, tag="yp", name="yp")
                            for k8 in range(8):
                                nc.tensor.matmul(
                                    yp[:, :CH],
                                    W2[br][:, 256 * k8 + 128 * m2:256 * k8 + 128 * m2 + 128],
                                    ht[:, k8, :],
                                    start=(k8 == 0), stop=(k8 == 7),
                                )
                            t1 = SB2.tile([128, CH], bf16, tag=f"t1_{m2}",
                                          name=f"t1_{m2}")
                            nc.vector.scalar_tensor_tensor(
                                t1[:, :], yp[:, :CH], B2[br][:, m2:m2 + 1],
                                otq[:, :, 64 * m2:64 * m2 + 64],
                                op0=mybir.AluOpType.add, op1=mybir.AluOpType.add)
                            rr = SBR.tile([128, CH], bf16, tag=f"res_{q}_{m2}",
                                          name=f"res_{q}_{m2}")
                            nc.gpsimd.tensor_add(rr[:, :], t1[:, :],
                                                 resT[m2][q][:, :])
                            res[(q, m2)] = rr
                    # ---- sub-B: LN over all chunks (DVE-only stats)
                    for q in range(NWG):
                        sq = [SB2.tile([128, CH], bf16, tag=f"sq_{m2}",
                                       name=f"sq_{m2}") for m2 in range(2)]
                        for m2 in range(2):
                            nc.scalar.square(sq[m2][:, :], res[(q, m2)][:, :])
                        sump = PS2.tile([128, 512], f32, tag="sum", name="sum")
                        sqsp = PS2.tile([128, 512], f32, tag="sqs", name="sqs")
                        for m2 in range(2):
                            nc.tensor.matmul(sump[:, :CH], ONE[:, :],
                                             res[(q, m2)][:, :],
                                             start=(m2 == 0), stop=(m2 == 1))
                            nc.tensor.matmul(sqsp[:, :CH], ONE[:, :], sq[m2][:, :],
                                             start=(m2 == 0), stop=(m2 == 1))
                        mean = SB2.tile([128, CH], bf16, tag="mean", name="mean")
                        nc.scalar.mul(mean[:, :], sump[:, :CH], 1.0 / EMBED)
                        sq2 = SB2.tile([128, CH], bf16, tag="sq2", name="sq2")
                        nc.scalar.mul(sq2[:, :], sqsp[:, :CH], 1.0 / EMBED)
                        m2t = SB2.tile([128, CH], bf16, tag="m2t", name="m2t")
                        nc.vector.tensor_mul(m2t[:, :], mean[:, :], mean[:, :])
                        dd = SB2.tile([128, CH], bf16, tag="dd", name="dd")
                        nc.vector.tensor_sub(dd[:, :], sq2[:, :], m2t[:, :])
                        vv = SB2.tile([128, CH], bf16, tag="vv", name="vv")
                        nc.vector.tensor_scalar_add(vv[:, :], dd[:, :], EPS[:, :])
                        y0 = SB2.tile([128, CH], bf16, tag="y0", name="y0")
                        nc.vector.tensor_scalar(
                            y0[:, :], vv[:, :], -0.155, 1.05,
                            op0=mybir.AluOpType.mult, op1=mybir.AluOpType.add)
                        yc = y0
                        for it in range(1):
                            tt_ = SB2.tile([128, CH], bf16, tag=f"nt{it}",
                                           name=f"nt{it}")
                            nc.vector.tensor_mul(tt_[:, :], yc[:, :], yc[:, :])
                            ss_ = SB2.tile([128, CH], bf16, tag=f"ns{it}",
                                           name=f"ns{it}")
                            nc.vector.scalar_tensor_tensor(
                                ss_[:, :], vv[:, :], -0.5, tt_[:, :],
                                op0=mybir.AluOpType.mult, op1=mybir.AluOpType.mult)
                            yn = SB2.tile([128, CH], bf16, tag=f"ny{it}",
                                          name=f"ny{it}")
                            nc.vector.scalar_tensor_tensor(
                                yn[:, :], ss_[:, :], 1.5, yc[:, :],
                                op0=mybir.AluOpType.add, op1=mybir.AluOpType.mult)
                            yc = yn
                        rstd = yc
                        mrstd = SB2.tile([128, CH], bf16, tag="mrstd", name="mrstd")
                        nc.vector.tensor_mul(mrstd[:, :], mean[:, :], rstd[:, :])
                        for m2 in range(2):
                            ta = SB2.tile([128, CH], bf16, tag=f"ta_{m2}",
                                          name=f"ta_{m2}")
                            nc.vector.scalar_tensor_tensor(
                                ta[:, :], res[(q, m2)][:, :], LNG[br][:, m2:m2 + 1],
                                rstd[:, :],
                                op0=mybir.AluOpType.mult, op1=mybir.AluOpType.mult)
                            tb = SB2.tile([128, CH], bf16, tag=f"tb_{m2}",
                                          name=f"tb_{m2}")
                            nc.vector.tensor_scalar(
                                tb[:, :], mrstd[:, :], LNG[br][:, m2:m2 + 1],
                                LNB[br][:, m2:m2 + 1],
                                op0=mybir.AluOpType.mult,
                                op1=mybir.AluOpType.subtract)
                            y = SB2.tile([128, CH], bf16, tag=f"y_{m2}",
                                         name=f"y_{m2}")
                            nc.vector.tensor_sub(y[:, :], ta[:, :], tb[:, :])
                            nc.sync.dma_start(
                                out=x_d[br][128 * m2:128 * m2 + 128,
                                            CH * q:CH * (q + 1)],
                                in_=y[:, :])
    if split_waits:
        _split_waits(nc)
    return nc


# ---------------------------------------------------------------- host side
def _win_part(x, ws):
    B, H, W, C = x.shape
    x = x.reshape(B, H // ws, ws, W // ws, ws, C)
    return x.transpose(0, 1, 3, 2, 4, 5).reshape(-1, ws * ws, C)


def _win_unpart(wins, ws, B, H, W):
    C = wins.shape[-1]
    x = wins.reshape(B, H // ws, W // ws, ws, ws, C)
    return x.transpose(0, 1, 3, 2, 4, 5).reshape(B, H, W, C)


def make_core_inputs(r_c, g_c, weights, NW):
    """r_c, g_c: [NW, 64, C] f32 -> in_map dict for one core."""
    import ml_dtypes
    bf = ml_dtypes.bfloat16
    T = NW * N

    def featmaj(a):  # [NW, 64, C] -> [C, T]
        return np.ascontiguousarray(
            a.transpose(2, 0, 1).reshape(EMBED, T)).astype(bf)

    def tokmaj(a):  # [NW, 64, C] -> [128, NW*128]
        t = a.reshape(NW, N, 2, 128).transpose(2, 1, 0, 3)  # [fb, tok, w, cm]
        return np.ascontiguousarray(t.reshape(128, NW * 128)).astype(bf)

    m = {
        "rT": featmaj(r_c), "gT": featmaj(g_c),
        "rtok": tokmaj(r_c), "gtok": tokmaj(g_c),
        "onesbd": np.kron(np.eye(2, dtype=np.float32),
                          np.ones((64, 64), np.float32)).astype(bf),
        "ones": np.ones((128, 128), np.float32).astype(bf),
    }
    for b in range(2):
        w1, b1, w2, b2, lng, lnb = weights[b]
        m[f"w1_{b}"] = np.ascontiguousarray(w1).astype(bf)
        m[f"w2_{b}"] = np.ascontiguousarray(w2.reshape(8, 128, EMBED)).astype(bf)
        m[f"b1_{b}"] = np.ascontiguousarray(b1.reshape(8, 128).T).astype(np.float32)
        m[f"b2_{b}"] = np.ascontiguousarray(b2.reshape(2, 128).T).astype(np.float32)
        m[f"lng_{b}"] = np.ascontiguousarray(lng.reshape(2, 128).T).astype(np.float32)
        m[f"lnb_{b}"] = np.ascontiguousarray(lnb.reshape(2, 128).T).astype(np.float32)
    return m


def postprocess(res, NW):
    """res: per-core result dicts -> (x1, x2) full arrays [8, 64, 64, 256]."""
    outs = []
    for b in range(2):
        wins = np.concatenate([
            np.asarray(r[f"x{b+1}T"], np.float32)
            .reshape(EMBED, NW, N).transpose(1, 2, 0)
            for r in res], axis=0)
        outs.append(_win_unpart(wins, WS, 8, 64, 64))
    return tuple(outs)


def kernel(c1, c2, window_size, mlp1_fc1_w, mlp1_fc1_b, mlp1_fc2_w, mlp1_fc2_b,
           ln1_g, ln1_b, mlp2_fc1_w, mlp2_fc1_b, mlp2_fc2_w, mlp2_fc2_b,
           ln2_g, ln2_b):
    from concourse.bass_utils import run_bass_kernel_spmd

    ws = int(window_size)
    assert ws == WS
    c1 = np.asarray(c1, np.float32)
    c2 = np.asarray(c2, np.float32)
    B, H, W, C = c1.shape
    r = _win_part(c1, ws)
    g = _win_part(c2, ws)
    n_win = r.shape[0]
    NW = n_win // 8
    weights = [
        (np.asarray(mlp1_fc1_w, np.float32), np.asarray(mlp1_fc1_b, np.float32),
         np.asarray(mlp1_fc2_w, np.float32), np.asarray(mlp1_fc2_b, np.float32),
         np.asarray(ln1_g, np.float32), np.asarray(ln1_b, np.float32)),
        (np.asarray(mlp2_fc1_w, np.float32), np.asarray(mlp2_fc1_b, np.float32),
         np.asarray(mlp2_fc2_w, np.float32), np.asarray(mlp2_fc2_b, np.float32),
         np.asarray(ln2_g, np.float32), np.asarray(ln2_b, np.float32)),
    ]
    if NW not in _CACHE:
        _CACHE[NW] = build_nc(NW)
    nc = _CACHE[NW]
    in_maps = [make_core_inputs(r[NW * c:NW * (c + 1)], g[NW * c:NW * (c + 1)],
                                weights, NW) for c in range(8)]
    res = run_bass_kernel_spmd(nc, in_maps, list(range(8))).results
    return postprocess(res, NW)
